# revision 6
# baseline (speedup 1.0000x reference)
"""Trainium2 Bass kernel for CFGNodeEncoderExpressionUpdateLayer.

Reference computation (per masked node row):
    idx  = nonzero(mask)                     # M rows of N
    prev = previous_cfg_nodes_encodings[idx]            # [M, 256]
    upd  = cfg_combined_expressions_encodings           # [M, 512]
    g    = sigmoid(concat(prev, upd) @ W_forget + b_forget)
    out_rows = g * prev + (1 - g) * (upd @ W_proj + b_proj)
    out  = previous_cfg_nodes_encodings; out[idx] = out_rows

Strategy:
  - Host: gather the M masked prev rows, shard rows across 8 cores
    (M/8 = 25000 rows each), transpose shards to [features, rows] so the
    device does only contiguous DMA and zero on-device transposes, pad
    rows to a multiple of 512.
  - Device (per core): for each 512-row block, accumulate the two GEMMs
    over 128-deep K chunks in PSUM (fp32r matmuls: full fp32 data at
    1 cycle/row), sigmoid+bias on ScalarE, and the gated blend
    out = pb - g*(pb - prev) with pb = proj + b_proj on VectorE using
    fused scalar_tensor_tensor ops.  Everything double/triple buffered
    by Tile.
  - Host: transpose shards back and scatter into a copy of the full
    prev tensor.  Unmasked rows never touch the device.
"""

import numpy as np

N_NODES = 400000
M_ROWS = 200000
D_NODE = 256
D_EXPR = 512
D_CAT = D_NODE + D_EXPR

N_CORES = 8
MC = M_ROWS // N_CORES          # masked rows per core
BLK = 512                       # rows per block (matmul moving dim)
NBLK = -(-MC // BLK)            # 49
MC_PAD = NBLK * BLK             # 25088

_CACHE = {}


def _build_nc(nblk=NBLK, repeat=1):
    import concourse.bacc as bacc
    import concourse.mybir as mybir
    import concourse.tile as tile

    f32 = mybir.dt.float32
    f32r = mybir.dt.float32r
    ALU = mybir.AluOpType

    mc_pad = nblk * BLK
    nc = bacc.Bacc("TRN2", target_bir_lowering=False, debug=False,
                   num_devices=N_CORES)

    xt_prev = nc.dram_tensor("xt_prev", [D_NODE, mc_pad], f32r,
                             kind="ExternalInput").ap()
    xt_upd = nc.dram_tensor("xt_upd", [D_EXPR, mc_pad], f32r,
                            kind="ExternalInput").ap()
    w_forget = nc.dram_tensor("w_forget", [D_CAT, D_NODE], f32r,
                              kind="ExternalInput").ap()
    b_forget = nc.dram_tensor("b_forget", [D_NODE], f32,
                              kind="ExternalInput").ap()
    w_proj = nc.dram_tensor("w_proj", [D_EXPR, D_NODE], f32r,
                            kind="ExternalInput").ap()
    b_proj = nc.dram_tensor("b_proj", [D_NODE], f32,
                            kind="ExternalInput").ap()
    out_t = nc.dram_tensor("out_t", [D_NODE, mc_pad], f32,
                           kind="ExternalOutput").ap()

    KP = D_NODE // 128   # 2 prev K-chunks
    KU = D_EXPR // 128   # 4 upd K-chunks
    KG = KP + KU         # 6 gate K-chunks
    NM = D_NODE // 128   # 2 output-feature chunks

    with tile.TileContext(nc) as tc:
        with (
            tc.tile_pool(name="wpool", bufs=1) as wpool,
            tc.tile_pool(name="io", bufs=3) as io,
            tc.tile_pool(name="mid", bufs=3) as mid,
            tc.tile_pool(name="psum", bufs=2, space="PSUM") as pp,
        ):
            # --- preload weights & biases (stay resident in SBUF) ---
            wf_t = {}
            for kc in range(KG):
                for m in range(NM):
                    t = wpool.tile([128, 128], f32r, tag=f"wf_{kc}_{m}")
                    nc.sync.dma_start(
                        t[:], w_forget[kc * 128:(kc + 1) * 128,
                                       m * 128:(m + 1) * 128])
                    wf_t[kc, m] = t
            wp_t = {}
            for kc in range(KU):
                for m in range(NM):
                    t = wpool.tile([128, 128], f32r, tag=f"wp_{kc}_{m}")
                    nc.sync.dma_start(
                        t[:], w_proj[kc * 128:(kc + 1) * 128,
                                     m * 128:(m + 1) * 128])
                    wp_t[kc, m] = t
            bf_t, bp_t = {}, {}
            for m in range(NM):
                t = wpool.tile([128, 1], f32, tag=f"bf_{m}")
                nc.sync.dma_start(t[:], b_forget[m * 128:(m + 1) * 128]
                                  .unsqueeze(1))
                bf_t[m] = t
                t = wpool.tile([128, 1], f32, tag=f"bp_{m}")
                nc.sync.dma_start(t[:], b_proj[m * 128:(m + 1) * 128]
                                  .unsqueeze(1))
                bp_t[m] = t

            # --- main loop over 512-row blocks ---
            for b in range(nblk * repeat):
                b = b % nblk
                s = b * BLK
                pv = []
                for c in range(KP):
                    t = io.tile([128, BLK], f32r, tag=f"pv{c}")
                    nc.sync.dma_start(
                        t[:], xt_prev[c * 128:(c + 1) * 128, s:s + BLK])
                    pv.append(t)
                up = []
                for c in range(KU):
                    t = io.tile([128, BLK], f32r, tag=f"up{c}")
                    nc.sync.dma_start(
                        t[:], xt_upd[c * 128:(c + 1) * 128, s:s + BLK])
                    up.append(t)
                rhs_gate = pv + up
                for m in range(NM):
                    pg = pp.tile([128, BLK], f32, tag=f"pg{m}")
                    for kc in range(KG):
                        nc.tensor.matmul(
                            pg[:], wf_t[kc, m][:], rhs_gate[kc][:],
                            start=(kc == 0), stop=(kc == KG - 1))
                    pj = pp.tile([128, BLK], f32, tag=f"pp{m}")
                    for kc in range(KU):
                        nc.tensor.matmul(
                            pj[:], wp_t[kc, m][:], up[kc][:],
                            start=(kc == 0), stop=(kc == KU - 1))
                    # g = sigmoid(gate_logits + b_forget)
                    g = mid.tile([128, BLK], f32, tag=f"g{m}")
                    nc.scalar.activation(
                        g[:], pg[:], mybir.ActivationFunctionType.Sigmoid,
                        bias=bf_t[m][:])
                    # e = (proj + b_proj) - prev
                    e = mid.tile([128, BLK], f32, tag=f"e{m}")
                    nc.vector.scalar_tensor_tensor(
                        e[:], pj[:], bp_t[m][:], pv[m][:].bitcast(f32),
                        op0=ALU.add, op1=ALU.subtract)
                    # t2 = g * e
                    t2 = mid.tile([128, BLK], f32, tag=f"t{m}")
                    nc.vector.tensor_mul(t2[:], g[:], e[:])
                    # out = (proj + b_proj) - t2 = g*prev + (1-g)*(proj+b_proj)
                    o = mid.tile([128, BLK], f32, tag=f"o{m}")
                    nc.vector.scalar_tensor_tensor(
                        o[:], pj[:], bp_t[m][:], t2[:],
                        op0=ALU.add, op1=ALU.subtract)
                    nc.sync.dma_start(
                        out_t[m * 128:(m + 1) * 128, s:s + BLK], o[:])

    nc.compile()
    return nc


def _get_nc(nblk=NBLK, repeat=1):
    key = (nblk, repeat)
    if key not in _CACHE:
        _CACHE[key] = _build_nc(nblk, repeat)
    return _CACHE[key]


def _prep_core_inputs(prev, upd, idx, wf, bf, wp, bp):
    """Build the 8 per-core input maps (host-side shard + transpose)."""
    in_maps = []
    for c in range(N_CORES):
        rows = idx[c * MC:(c + 1) * MC]
        xt_prev = np.zeros((D_NODE, MC_PAD), np.float32)
        xt_prev[:, :MC] = prev[rows].T
        xt_upd = np.zeros((D_EXPR, MC_PAD), np.float32)
        xt_upd[:, :MC] = upd[c * MC:(c + 1) * MC].T
        in_maps.append({
            "xt_prev": np.ascontiguousarray(xt_prev),
            "xt_upd": np.ascontiguousarray(xt_upd),
            "w_forget": wf,
            "b_forget": bf,
            "w_proj": wp,
            "b_proj": bp,
        })
    return in_maps


def _run_spmd(in_maps, trace=False):
    from concourse.bass_utils import run_bass_kernel_spmd
    nc = _get_nc()
    return run_bass_kernel_spmd(nc, in_maps, core_ids=list(range(N_CORES)),
                                trace=trace)


def kernel(**inputs):
    prev = np.ascontiguousarray(
        np.asarray(inputs["previous_cfg_nodes_encodings"], np.float32))
    upd = np.ascontiguousarray(
        np.asarray(inputs["cfg_combined_expressions_encodings"], np.float32))
    mask = np.asarray(inputs["cfg_nodes_has_expression_mask"], bool)
    wf = np.ascontiguousarray(np.asarray(inputs["W_forget"], np.float32))
    bf = np.ascontiguousarray(np.asarray(inputs["b_forget"], np.float32))
    wp = np.ascontiguousarray(np.asarray(inputs["W_proj"], np.float32))
    bp = np.ascontiguousarray(np.asarray(inputs["b_proj"], np.float32))

    # mimic jnp.nonzero(mask, size=M, fill_value=0)
    idx = np.flatnonzero(mask)
    if idx.size >= M_ROWS:
        idx = idx[:M_ROWS]
    else:
        idx = np.concatenate(
            [idx, np.zeros(M_ROWS - idx.size, idx.dtype)])

    in_maps = _prep_core_inputs(prev, upd, idx, wf, bf, wp, bp)
    results = _run_spmd(in_maps).results

    out = prev.copy()
    for c in range(N_CORES):
        rows = idx[c * MC:(c + 1) * MC]
        out[rows] = results[c]["out_t"][:, :MC].T
    return out


# revision 11
# speedup vs baseline: 2.9752x; 2.9752x over previous
"""Trainium2 Bass kernel for CFGNodeEncoderExpressionUpdateLayer.

Reference computation (per masked node row):
    idx  = nonzero(mask)                     # M rows of N
    prev = previous_cfg_nodes_encodings[idx]            # [M, 256]
    upd  = cfg_combined_expressions_encodings           # [M, 512]
    g    = sigmoid(concat(prev, upd) @ W_forget + b_forget)
    out_rows = g * prev + (1 - g) * (upd @ W_proj + b_proj)
    out  = previous_cfg_nodes_encodings; out[idx] = out_rows

Strategy:
  - Host: gather the M masked prev rows, shard rows across 8 cores
    (M/8 = 25000 rows each), transpose shards to [features, rows] so the
    device does only contiguous DMA and zero on-device transposes, pad
    rows to a multiple of 512.
  - Device (per core): for each 512-row block, accumulate the two GEMMs
    over 128-deep K chunks in PSUM (fp32r matmuls: full fp32 data at
    1 cycle/row), sigmoid+bias on ScalarE, and the gated blend
    out = pb - g*(pb - prev) with pb = proj + b_proj on VectorE using
    fused scalar_tensor_tensor ops.  Everything double/triple buffered
    by Tile.
  - Host: transpose shards back and scatter into a copy of the full
    prev tensor.  Unmasked rows never touch the device.
"""

import numpy as np

N_NODES = 400000
M_ROWS = 200000
D_NODE = 256
D_EXPR = 512
D_CAT = D_NODE + D_EXPR

N_CORES = 8
MC = M_ROWS // N_CORES          # masked rows per core
BLK = 512                       # rows per block (matmul moving dim)
NBLK = -(-MC // BLK)            # 49
MC_PAD = NBLK * BLK             # 25088

_CACHE = {}


def _build_nc(nblk=NBLK, repeat=1, sup=2048, io_bufs=2, mid_bufs=3,
              psum_bufs=2):
    import concourse.bacc as bacc
    import concourse.mybir as mybir
    import concourse.tile as tile

    f32 = mybir.dt.float32
    f32r = mybir.dt.float32r
    ALU = mybir.AluOpType

    mc_pad = nblk * BLK
    nc = bacc.Bacc("TRN2", target_bir_lowering=False, debug=False,
                   num_devices=N_CORES)

    xt_prev = nc.dram_tensor("xt_prev", [D_NODE, mc_pad], f32r,
                             kind="ExternalInput").ap()
    xt_upd = nc.dram_tensor("xt_upd", [D_EXPR, mc_pad], f32r,
                            kind="ExternalInput").ap()
    w_forget = nc.dram_tensor("w_forget", [D_CAT, D_NODE], f32r,
                              kind="ExternalInput").ap()
    b_forget = nc.dram_tensor("b_forget", [D_NODE], f32,
                              kind="ExternalInput").ap()
    w_proj = nc.dram_tensor("w_proj", [D_EXPR, D_NODE], f32r,
                            kind="ExternalInput").ap()
    b_proj = nc.dram_tensor("b_proj", [D_NODE], f32,
                            kind="ExternalInput").ap()
    out_t = nc.dram_tensor("out_t", [D_NODE, mc_pad], f32,
                           kind="ExternalOutput").ap()

    KP = D_NODE // 128   # 2 prev K-chunks
    KU = D_EXPR // 128   # 4 upd K-chunks
    KG = KP + KU         # 6 gate K-chunks
    NM = D_NODE // 128   # 2 output-feature chunks

    # column segments of the padded row space (big segments => big DMAs)
    mc_pad = nblk * BLK
    segs = []
    pos = 0
    while pos < mc_pad:
        w = min(sup, mc_pad - pos)
        segs.append((pos, w))
        pos += w

    with tile.TileContext(nc) as tc:
        with (
            tc.tile_pool(name="wpool", bufs=1) as wpool,
            tc.tile_pool(name="io", bufs=io_bufs) as io,
            tc.tile_pool(name="mid", bufs=mid_bufs) as mid,
            tc.tile_pool(name="psum", bufs=psum_bufs, space="PSUM") as pp,
        ):
            # --- preload weights & biases (stay resident in SBUF) ---
            wf_t = {}
            for kc in range(KG):
                for m in range(NM):
                    t = wpool.tile([128, 128], f32r, tag=f"wf_{kc}_{m}")
                    nc.sync.dma_start(
                        t[:], w_forget[kc * 128:(kc + 1) * 128,
                                       m * 128:(m + 1) * 128])
                    wf_t[kc, m] = t
            wp_t = {}
            for kc in range(KU):
                for m in range(NM):
                    t = wpool.tile([128, 128], f32r, tag=f"wp_{kc}_{m}")
                    nc.sync.dma_start(
                        t[:], w_proj[kc * 128:(kc + 1) * 128,
                                     m * 128:(m + 1) * 128])
                    wp_t[kc, m] = t
            bf_t, bp_t = {}, {}
            for m in range(NM):
                t = wpool.tile([128, 1], f32, tag=f"bf_{m}")
                nc.sync.dma_start(t[:], b_forget[m * 128:(m + 1) * 128]
                                  .unsqueeze(1))
                bf_t[m] = t
                t = wpool.tile([128, 1], f32, tag=f"bp_{m}")
                nc.sync.dma_start(t[:], b_proj[m * 128:(m + 1) * 128]
                                  .unsqueeze(1))
                bp_t[m] = t

            # --- main loop over column segments ---
            for it in range(len(segs) * repeat):
                s, w = segs[it % len(segs)]
                pv = []
                for c in range(KP):
                    t = io.tile([128, sup], f32r, tag=f"pv{c}")
                    nc.sync.dma_start(
                        t[:, :w], xt_prev[c * 128:(c + 1) * 128, s:s + w])
                    pv.append(t)
                up = []
                for c in range(KU):
                    t = io.tile([128, sup], f32r, tag=f"up{c}")
                    nc.sync.dma_start(
                        t[:, :w], xt_upd[c * 128:(c + 1) * 128, s:s + w])
                    up.append(t)
                ot = [io.tile([128, sup], f32, tag=f"o{m}", name=f"ot{m}")
                      for m in range(NM)]
                rhs_gate = pv + up
                for j in range(w // BLK):
                    js = slice(j * BLK, (j + 1) * BLK)
                    for m in range(NM):
                        pg = pp.tile([128, BLK], f32, tag=f"pg{m}")
                        for kc in range(KG):
                            nc.tensor.matmul(
                                pg[:], wf_t[kc, m][:], rhs_gate[kc][:, js],
                                start=(kc == 0), stop=(kc == KG - 1))
                        pj = pp.tile([128, BLK], f32, tag=f"pp{m}")
                        for kc in range(KU):
                            nc.tensor.matmul(
                                pj[:], wp_t[kc, m][:], up[kc][:, js],
                                start=(kc == 0), stop=(kc == KU - 1))
                        # g = sigmoid(gate_logits + b_forget)
                        g = mid.tile([128, BLK], f32, tag=f"g{m}")
                        nc.scalar.activation(
                            g[:], pg[:],
                            mybir.ActivationFunctionType.Sigmoid,
                            bias=bf_t[m][:])
                        # e = (proj + b_proj) - prev
                        e = mid.tile([128, BLK], f32, tag=f"e{m}")
                        nc.vector.scalar_tensor_tensor(
                            e[:], pj[:], bp_t[m][:],
                            pv[m][:, js].bitcast(f32),
                            op0=ALU.add, op1=ALU.subtract)
                        # t2 = g * e
                        t2 = mid.tile([128, BLK], f32, tag=f"t{m}")
                        nc.vector.tensor_mul(t2[:], g[:], e[:])
                        # out = (proj+b_proj) - t2 = g*prev + (1-g)*(proj+b_p)
                        nc.vector.scalar_tensor_tensor(
                            ot[m][:, js], pj[:], bp_t[m][:], t2[:],
                            op0=ALU.add, op1=ALU.subtract)
                for m in range(NM):
                    nc.sync.dma_start(
                        out_t[m * 128:(m + 1) * 128, s:s + w], ot[m][:, :w])

    nc.compile()
    return nc


def _get_nc(nblk=NBLK, repeat=1, **kw):
    key = (nblk, repeat, tuple(sorted(kw.items())))
    if key not in _CACHE:
        _CACHE[key] = _build_nc(nblk, repeat, **kw)
    return _CACHE[key]


def _prep_core_inputs(prev, upd, idx, wf, bf, wp, bp):
    """Build the 8 per-core input maps (host-side shard + transpose)."""
    in_maps = []
    for c in range(N_CORES):
        rows = idx[c * MC:(c + 1) * MC]
        xt_prev = np.zeros((D_NODE, MC_PAD), np.float32)
        xt_prev[:, :MC] = prev[rows].T
        xt_upd = np.zeros((D_EXPR, MC_PAD), np.float32)
        xt_upd[:, :MC] = upd[c * MC:(c + 1) * MC].T
        in_maps.append({
            "xt_prev": np.ascontiguousarray(xt_prev),
            "xt_upd": np.ascontiguousarray(xt_upd),
            "w_forget": wf,
            "b_forget": bf,
            "w_proj": wp,
            "b_proj": bp,
        })
    return in_maps


def _run_spmd(in_maps, trace=False):
    from concourse.bass_utils import run_bass_kernel_spmd
    nc = _get_nc()
    return run_bass_kernel_spmd(nc, in_maps, core_ids=list(range(N_CORES)),
                                trace=trace)


def kernel(**inputs):
    prev = np.ascontiguousarray(
        np.asarray(inputs["previous_cfg_nodes_encodings"], np.float32))
    upd = np.ascontiguousarray(
        np.asarray(inputs["cfg_combined_expressions_encodings"], np.float32))
    mask = np.asarray(inputs["cfg_nodes_has_expression_mask"], bool)
    wf = np.ascontiguousarray(np.asarray(inputs["W_forget"], np.float32))
    bf = np.ascontiguousarray(np.asarray(inputs["b_forget"], np.float32))
    wp = np.ascontiguousarray(np.asarray(inputs["W_proj"], np.float32))
    bp = np.ascontiguousarray(np.asarray(inputs["b_proj"], np.float32))

    # mimic jnp.nonzero(mask, size=M, fill_value=0)
    idx = np.flatnonzero(mask)
    if idx.size >= M_ROWS:
        idx = idx[:M_ROWS]
    else:
        idx = np.concatenate(
            [idx, np.zeros(M_ROWS - idx.size, idx.dtype)])

    in_maps = _prep_core_inputs(prev, upd, idx, wf, bf, wp, bp)
    results = _run_spmd(in_maps).results

    out = prev.copy()
    for c in range(N_CORES):
        rows = idx[c * MC:(c + 1) * MC]
        out[rows] = results[c]["out_t"][:, :MC].T
    return out
